# revision 2
# baseline (speedup 1.0000x reference)
"""ContraNorm kernel for 8 Trainium2 NeuronCores — v2 (all-gather + on-device mask).

Math (reference):
    norm_x = x / max(||x||_row, eps)
    sim    = (norm_x @ norm_x.T) / tau          # [N, N], tau = 1
    sim[edge_index[0], edge_index[1]] = -inf
    attn   = softmax(sim, axis=1)
    out    = 1.1 * x - 0.1 * (attn @ x)

Distribution: core k owns output rows [k*1024, (k+1)*1024) (contiguous
blocks, no rolling).  Each core receives ONLY its own row shard of x (bf16)
plus compact int16 scatter-index lists for its slice of the edge mask —
~0.9 MiB per core instead of the 22 MiB (full x + dense mask) of v1.

On device, each core normalizes + PE-transposes its own shard, then one
AllGather distributes [nxT_k | x_k] blocks to every core.  The c-side
operands (normalized-transposed nxT and the V-matmul rhs [x | 1]) are
unpacked from the gathered buffer with strided DMAs.

The -inf edge mask is applied post-exp: gpsimd local_scatter builds a
{0,-1} indicator tile from the per-partition index lists, and one fused
DVE op computes et *= (ind + 1).

Because sim entries are cosine similarities in [-1, 1], softmax needs no
running max.  The row-sum comes from a ones-column appended to the V rhs.
"""

import numpy as np
import ml_dtypes

N = 8192          # rows of x
D = 256           # features
P = 128           # SBUF partitions
R = N // 8        # 1024 rows per core
NT = N // P       # 64 c-chunks
RT = R // P       # 8 own m-chunks
HALF = 512        # m columns per pass
SCALE = 0.1
NCORES = 8

_prog_cache = {}


def _build_program(kpad):
    import concourse.bacc as bacc
    import concourse.tile as tile
    from concourse import mybir
    from concourse.masks import make_identity
    from contextlib import ExitStack

    f32 = mybir.dt.float32
    bf16 = mybir.dt.bfloat16
    i16 = mybir.dt.int16
    ADD = mybir.AluOpType.add
    MUL = mybir.AluOpType.mult
    Exp = mybir.ActivationFunctionType.Exp
    Sqrt = mybir.ActivationFunctionType.Sqrt

    nc = bacc.Bacc("TRN2", target_bir_lowering=False, debug=False,
                   num_devices=NCORES)

    xk_h = nc.dram_tensor("xk", [R, D], bf16, kind="ExternalInput")
    si_h = nc.dram_tensor("si", [P, 2 * NT, kpad], i16, kind="ExternalInput")
    out_h = nc.dram_tensor("out", [R, D], f32, kind="ExternalOutput")

    xk_d = xk_h.ap().rearrange("(j p) d -> p j d", p=P)       # [128, 8, 256]
    xk_flat = xk_h.ap().rearrange("(a b) d -> a (b d)", b=4)  # [256, 1024]
    out_d = out_h.ap()

    with ExitStack() as ctx:
        tc = ctx.enter_context(tile.TileContext(nc))

        dram = ctx.enter_context(tc.tile_pool(name="dram", bufs=1,
                                              space="DRAM"))
        consts = ctx.enter_context(tc.tile_pool(name="consts", bufs=1))
        pre = ctx.enter_context(tc.tile_pool(name="pre", bufs=2))
        work = ctx.enter_context(tc.tile_pool(name="work", bufs=4))
        ps_t = ctx.enter_context(tc.tile_pool(name="ps_t", bufs=2, space="PSUM"))
        ps_s = ctx.enter_context(tc.tile_pool(name="ps_s", bufs=2, space="PSUM"))
        ps_v = ctx.enter_context(tc.tile_pool(name="ps_v", bufs=1, space="PSUM"))

        # gather payload per core: rows 0:256 = nxoT (row h*128+p),
        # rows 256:512 = xk flattened [256, 1024]
        gin = dram.tile([4 * P, 1024], bf16)
        gout = dram.tile([NCORES * 4 * P, 1024], bf16)

        xa = consts.tile([P, NT, D + 1], bf16)    # V rhs: [x | 1] per c-chunk
        nxT = consts.tile([P, 2, N], bf16)        # norm_x^T, c side (gathered)
        xk = consts.tile([P, RT, D], bf16)        # own raw rows
        nxo = consts.tile([P, RT, D], bf16)       # own normalized rows
        nxoT = consts.tile([P, 2, R], bf16)       # own norm_x^T (m side)
        si = consts.tile([P, 2 * NT, kpad], i16)  # mask scatter indices
        mone = consts.tile([P, kpad], bf16)
        ident = consts.tile([P, P], bf16)
        ssq = consts.tile([P, RT], f32)
        inv = consts.tile([P, RT], f32)

        make_identity(nc, ident)
        nc.sync.dma_start(out=xk, in_=xk_d)
        nc.sync.dma_start(out=si, in_=si_h.ap())
        nc.vector.memset(mone, -1.0)
        nc.vector.memset(xa[:, :, D:D + 1], 1.0)

        # ---- own-row norms + normalized rows ----
        for j in range(RT):
            sq = pre.tile([P, D], bf16, tag="sq")
            nc.vector.scalar_tensor_tensor(
                out=sq, in0=xk[:, j], scalar=1.0, in1=xk[:, j],
                op0=MUL, op1=MUL, accum_out=ssq[:, j:j + 1],
            )
        std = pre.tile([P, RT], f32, tag="std")
        nc.scalar.activation(std, ssq, Sqrt)
        nc.vector.reciprocal(inv, std)
        for j in range(RT):
            nc.vector.tensor_scalar_mul(nxo[:, j], xk[:, j], inv[:, j:j + 1])

        # ---- transpose own normalized rows -> nxoT [d, m] ----
        for h in range(2):
            for q in range(2):
                tp4 = ps_t.tile([P, 4, P], bf16, tag="tp4")
                for jj in range(4):
                    nc.tensor.transpose(
                        tp4[:, jj], nxo[:, q * 4 + jj, h * P:(h + 1) * P],
                        ident)
                nc.vector.tensor_copy(
                    out=nxoT[:, h, q * 4 * P:(q + 1) * 4 * P], in_=tp4)

        # ---- stage + all-gather ----
        gin_A = gin[0:2 * P, :].rearrange("(h p) m -> p h m", h=2)
        nc.sync.dma_start(out=gin_A, in_=nxoT)
        nc.sync.dma_start(out=gin[2 * P:4 * P, :], in_=xk_flat)
        nc.gpsimd.collective_compute(
            "AllGather", mybir.AluOpType.bypass,
            replica_groups=[list(range(NCORES))],
            ins=[gin.opt()], outs=[gout.opt()],
        )

        # ---- unpack gathered blocks: nxT (c side) + xa (V rhs) ----
        g_nx = gout.rearrange("(k s h p) m -> s p h k m",
                              k=NCORES, s=2, h=2, p=P)[0]
        g_x = gout.rearrange("(k s j q) (o d) -> (q o) s k j d",
                             k=NCORES, s=2, j=8, q=32, o=4, d=D)
        for k in range(NCORES):
            nc.sync.dma_start(out=nxT[:, :, k * R:(k + 1) * R],
                              in_=g_nx[:, :, k])
            nc.sync.dma_start(out=xa[:, k * 8:(k + 1) * 8, 0:D],
                              in_=g_x[:, 1, k])

        # ---- main: two passes over this core's 1024 rows (512 each) ----
        for half in range(2):
            m0 = half * HALF
            pv = [
                ps_v.tile([P, D + 1], f32, tag=f"pv{i}", name=f"pv{i}")
                for i in range(4)
            ]
            for t in range(NT):
                ps = ps_s.tile([P, HALF], f32, tag="ps")
                nc.tensor.matmul(ps, nxT[:, 0, t * P:(t + 1) * P],
                                 nxoT[:, 0, m0:m0 + HALF],
                                 start=True, stop=False)
                nc.tensor.matmul(ps, nxT[:, 1, t * P:(t + 1) * P],
                                 nxoT[:, 1, m0:m0 + HALF],
                                 start=False, stop=True)
                ind = work.tile([P, HALF], bf16, tag="ind")
                nc.gpsimd.local_scatter(
                    out_ap=ind, data_ap=mone,
                    idxs_ap=si[:, half * NT + t, :],
                    channels=P, num_elems=HALF, num_idxs=kpad,
                )
                et = work.tile([P, HALF], bf16, tag="et", bufs=6)
                nc.scalar.activation(et, ps, Exp)
                nc.vector.scalar_tensor_tensor(
                    out=et, in0=ind, scalar=1.0, in1=et, op0=ADD, op1=MUL)
                for mi in range(4):
                    nc.tensor.matmul(
                        pv[mi], et[:, mi * P:(mi + 1) * P], xa[:, t, :],
                        start=(t == 0), stop=(t == NT - 1))
            for mi in range(4):
                jj = half * 4 + mi
                sinv = work.tile([P, 1], f32, tag="sinv")
                nc.vector.reciprocal(sinv, pv[mi][:, D:D + 1])
                res = work.tile([P, D], f32, tag="res")
                nc.vector.tensor_scalar(
                    out=res, in0=pv[mi][:, 0:D],
                    scalar1=sinv, scalar2=-SCALE, op0=MUL, op1=MUL)
                t1 = work.tile([P, D], f32, tag="t1")
                nc.vector.tensor_scalar_mul(t1, xk[:, jj], 1.0 + SCALE)
                nc.vector.tensor_add(res, res, t1)
                nc.sync.dma_start(out=out_d[jj * P:(jj + 1) * P, :], in_=res)

    nc.compile()
    return nc


def get_program(kpad):
    if kpad not in _prog_cache:
        _prog_cache[kpad] = _build_program(kpad)
    return _prog_cache[kpad]


def _prep_mask(edge_index):
    """Per-core int16 scatter-index lists for the edge mask.

    si[k][p, half*NT + t, slot] = m offset inside the 512-wide half-window,
    for each deduplicated edge (r, c) with r in core k's rows, c = t*128+p.
    Unused slots are -1 (ignored by local_scatter).
    """
    ei = np.asarray(edge_index)
    r = ei[0].astype(np.int64)
    c = ei[1].astype(np.int64)
    ok = (r >= 0) & (r < N) & (c >= 0) & (c < N)
    r, c = r[ok], c[ok]
    key = np.unique(r * N + c)
    r, c = key // N, key % N

    core = r // R
    half = (r % R) // HALF
    moff = (r % R) % HALF
    t = c // P
    p = c % P
    col = half * NT + t
    cell = (core * P + p) * (2 * NT) + col

    order = np.argsort(cell, kind="stable")
    cell_s = cell[order]
    moff_s = moff[order]
    idx = np.arange(len(cell_s))
    first = np.ones(len(cell_s), dtype=bool)
    if len(cell_s) > 1:
        first[1:] = cell_s[1:] != cell_s[:-1]
    start = np.where(first, idx, 0)
    np.maximum.accumulate(start, out=start)
    slot = idx - start

    kmax = int(slot.max()) + 1 if len(slot) else 1
    kpad = max(2, (kmax + 1) // 2 * 2)

    si_all = np.full((NCORES, P, 2 * NT, kpad), -1, dtype=np.int16)
    core_s = cell_s // (P * 2 * NT)
    p_s = (cell_s // (2 * NT)) % P
    col_s = cell_s % (2 * NT)
    si_all[core_s, p_s, col_s, slot] = moff_s.astype(np.int16)
    return si_all, kpad


def make_in_maps(x, edge_index):
    x = np.asarray(x, dtype=np.float32)
    si_all, kpad = _prep_mask(edge_index)
    xbf = x.astype(ml_dtypes.bfloat16)
    in_maps = []
    for k in range(NCORES):
        in_maps.append({
            "xk": np.ascontiguousarray(xbf[k * R:(k + 1) * R]),
            "si": np.ascontiguousarray(si_all[k]),
        })
    return in_maps, kpad


def run(x, edge_index, trace=False):
    from concourse.bass_utils import run_bass_kernel_spmd

    in_maps, kpad = make_in_maps(x, edge_index)
    nc = get_program(kpad)
    br = run_bass_kernel_spmd(nc, in_maps, list(range(NCORES)), trace=trace)
    out = np.concatenate(
        [br.results[k]["out"] for k in range(NCORES)], axis=0
    ).astype(np.float32)
    return out, br


def kernel(x, edge_index):
    out, _ = run(x, edge_index, trace=False)
    return out


# revision 5
# speedup vs baseline: 1.0795x; 1.0795x over previous
"""ContraNorm kernel for 8 Trainium2 NeuronCores — v2 (all-gather + on-device mask).

Math (reference):
    norm_x = x / max(||x||_row, eps)
    sim    = (norm_x @ norm_x.T) / tau          # [N, N], tau = 1
    sim[edge_index[0], edge_index[1]] = -inf
    attn   = softmax(sim, axis=1)
    out    = 1.1 * x - 0.1 * (attn @ x)

Distribution: core k owns output rows [k*1024, (k+1)*1024) (contiguous
blocks, no rolling).  Each core receives ONLY its own row shard of x (bf16)
plus compact int16 scatter-index lists for its slice of the edge mask —
~0.9 MiB per core instead of the 22 MiB (full x + dense mask) of v1.

On device, each core normalizes + PE-transposes its own shard, then one
AllGather distributes [nxT_k | x_k] blocks to every core.  The c-side
operands (normalized-transposed nxT and the V-matmul rhs [x | 1]) are
unpacked from the gathered buffer with strided DMAs.

The -inf edge mask is applied post-exp: gpsimd local_scatter builds a
{0,-1} indicator tile from the per-partition index lists, and one fused
DVE op computes et *= (ind + 1).

Because sim entries are cosine similarities in [-1, 1], softmax needs no
running max.  The row-sum comes from a ones-column appended to the V rhs.
"""

import numpy as np
import ml_dtypes

N = 8192          # rows of x
D = 256           # features
P = 128           # SBUF partitions
R = N // 8        # 1024 rows per core
NT = N // P       # 64 c-chunks
RT = R // P       # 8 own m-chunks
HALF = 512        # m columns per pass
SCALE = 0.1
NCORES = 8

_prog_cache = {}


def _build_program(kpad):
    import concourse.bacc as bacc
    import concourse.tile as tile
    from concourse import mybir
    from concourse.masks import make_identity
    from contextlib import ExitStack

    f32 = mybir.dt.float32
    bf16 = mybir.dt.bfloat16
    i16 = mybir.dt.int16
    ADD = mybir.AluOpType.add
    MUL = mybir.AluOpType.mult
    Exp = mybir.ActivationFunctionType.Exp
    Sqrt = mybir.ActivationFunctionType.Sqrt

    nc = bacc.Bacc("TRN2", target_bir_lowering=False, debug=False,
                   num_devices=NCORES)

    xk_h = nc.dram_tensor("xk", [R, D], bf16, kind="ExternalInput")
    si_h = nc.dram_tensor("si", [P, 2 * NT, kpad], i16, kind="ExternalInput")
    out_h = nc.dram_tensor("out", [R, D], bf16, kind="ExternalOutput")

    xk_d = xk_h.ap().rearrange("(j p) d -> p j d", p=P)       # [128, 8, 256]
    xk_flat = xk_h.ap().rearrange("(a b) d -> a (b d)", b=4)  # [256, 1024]
    out_d = out_h.ap()

    with ExitStack() as ctx:
        tc = ctx.enter_context(tile.TileContext(nc))

        dram = ctx.enter_context(tc.tile_pool(name="dram", bufs=1,
                                              space="DRAM"))
        consts = ctx.enter_context(tc.tile_pool(name="consts", bufs=1))
        pre = ctx.enter_context(tc.tile_pool(name="pre", bufs=2))
        work = ctx.enter_context(tc.tile_pool(name="work", bufs=4))
        ps_t = ctx.enter_context(tc.tile_pool(name="ps_t", bufs=2, space="PSUM"))
        ps_s = ctx.enter_context(tc.tile_pool(name="ps_s", bufs=2, space="PSUM"))
        ps_v = ctx.enter_context(tc.tile_pool(name="ps_v", bufs=1, space="PSUM"))

        # gather payload per core: rows 0:256 = nxoT (row h*128+p),
        # rows 256:512 = xk flattened [256, 1024]
        gin = dram.tile([4 * P, 1024], bf16)
        gout = dram.tile([NCORES * 4 * P, 1024], bf16, addr_space="Shared")

        xa = consts.tile([P, NT, D + 1], bf16)    # V rhs: [x | 1] per c-chunk
        nxT = consts.tile([P, 2, N], bf16)        # norm_x^T, c side (gathered)
        xk = consts.tile([P, RT, D], bf16)        # own raw rows
        nxo = consts.tile([P, RT, D], bf16)       # own normalized rows
        nxoT = consts.tile([P, 2, R], bf16)       # own norm_x^T (m side)
        si = consts.tile([P, 2 * NT, kpad], i16)  # mask scatter indices
        mone = consts.tile([P, kpad], bf16)
        ident = consts.tile([P, P], bf16)
        ssq = consts.tile([P, RT], f32)
        inv = consts.tile([P, RT], f32)

        make_identity(nc, ident)
        nc.sync.dma_start(out=xk, in_=xk_d)
        nc.sync.dma_start(out=si, in_=si_h.ap())
        nc.vector.memset(mone, -1.0)
        nc.vector.memset(xa[:, :, D:D + 1], 1.0)

        # ---- own-row norms + normalized rows ----
        for j in range(RT):
            sq = pre.tile([P, D], bf16, tag="sq")
            nc.vector.scalar_tensor_tensor(
                out=sq, in0=xk[:, j], scalar=1.0, in1=xk[:, j],
                op0=MUL, op1=MUL, accum_out=ssq[:, j:j + 1],
            )
        std = pre.tile([P, RT], f32, tag="std")
        nc.scalar.activation(std, ssq, Sqrt)
        nc.vector.reciprocal(inv, std)
        for j in range(RT):
            nc.vector.tensor_scalar_mul(nxo[:, j], xk[:, j], inv[:, j:j + 1])

        # ---- transpose own normalized rows -> nxoT [d, m] ----
        for h in range(2):
            for q in range(2):
                tp4 = ps_t.tile([P, 4, P], bf16, tag="tp4")
                for jj in range(4):
                    nc.tensor.transpose(
                        tp4[:, jj], nxo[:, q * 4 + jj, h * P:(h + 1) * P],
                        ident)
                nc.vector.tensor_copy(
                    out=nxoT[:, h, q * 4 * P:(q + 1) * 4 * P], in_=tp4)

        # ---- stage + all-gather ----
        gin_A = gin[0:2 * P, :].rearrange("(h p) m -> p h m", h=2)
        nc.sync.dma_start(out=gin_A, in_=nxoT)
        nc.sync.dma_start(out=gin[2 * P:4 * P, :], in_=xk_flat)
        nc.gpsimd.collective_compute(
            "AllGather", mybir.AluOpType.bypass,
            replica_groups=[list(range(NCORES))],
            ins=[gin.opt()], outs=[gout.opt()],
        )

        # ---- unpack gathered blocks: nxT (c side) + xa (V rhs) ----
        g_nx = gout.rearrange("(k s h p) m -> s p h k m",
                              k=NCORES, s=2, h=2, p=P)[0]
        g_x = gout.rearrange("(k s j q) (o d) -> (q o) s k j d",
                             k=NCORES, s=2, j=8, q=32, o=4, d=D)
        for k in range(NCORES):
            nc.sync.dma_start(out=nxT[:, :, k * R:(k + 1) * R],
                              in_=g_nx[:, :, k])
            nc.sync.dma_start(out=xa[:, k * 8:(k + 1) * 8, 0:D],
                              in_=g_x[:, 1, k])

        # ---- main: two passes over this core's 1024 rows (512 each) ----
        for half in range(2):
            m0 = half * HALF
            pv = [
                ps_v.tile([P, D + 1], f32, tag=f"pv{i}", name=f"pv{i}")
                for i in range(4)
            ]
            for t in range(NT):
                ps = ps_s.tile([P, HALF], f32, tag="ps")
                nc.tensor.matmul(ps, nxT[:, 0, t * P:(t + 1) * P],
                                 nxoT[:, 0, m0:m0 + HALF],
                                 start=True, stop=False)
                nc.tensor.matmul(ps, nxT[:, 1, t * P:(t + 1) * P],
                                 nxoT[:, 1, m0:m0 + HALF],
                                 start=False, stop=True)
                ind = work.tile([P, HALF], bf16, tag="ind")
                nc.gpsimd.local_scatter(
                    out_ap=ind, data_ap=mone,
                    idxs_ap=si[:, half * NT + t, :],
                    channels=P, num_elems=HALF, num_idxs=kpad,
                )
                et = work.tile([P, HALF], bf16, tag="et", bufs=6)
                nc.scalar.activation(et, ps, Exp)
                nc.vector.scalar_tensor_tensor(
                    out=et, in0=ind, scalar=1.0, in1=et, op0=ADD, op1=MUL)
                for mi in range(4):
                    nc.tensor.matmul(
                        pv[mi], et[:, mi * P:(mi + 1) * P], xa[:, t, :],
                        start=(t == 0), stop=(t == NT - 1))
            for mi in range(4):
                jj = half * 4 + mi
                sinv = work.tile([P, 1], f32, tag="sinv")
                nc.vector.reciprocal(sinv, pv[mi][:, D:D + 1])
                res = work.tile([P, D], f32, tag="res")
                nc.vector.tensor_scalar(
                    out=res, in0=pv[mi][:, 0:D],
                    scalar1=sinv, scalar2=-SCALE, op0=MUL, op1=MUL)
                t1 = work.tile([P, D], f32, tag="t1")
                nc.vector.tensor_scalar_mul(t1, xk[:, jj], 1.0 + SCALE)
                resb = work.tile([P, D], bf16, tag="resb")
                nc.vector.tensor_add(resb, res, t1)
                nc.sync.dma_start(out=out_d[jj * P:(jj + 1) * P, :], in_=resb)

    nc.compile()
    return nc


def get_program(kpad):
    if kpad not in _prog_cache:
        _prog_cache[kpad] = _build_program(kpad)
    return _prog_cache[kpad]


def _prep_mask(edge_index):
    """Per-core int16 scatter-index lists for the edge mask.

    si[k][p, half*NT + t, slot] = m offset inside the 512-wide half-window,
    for each deduplicated edge (r, c) with r in core k's rows, c = t*128+p.
    Unused slots are -1 (ignored by local_scatter).
    """
    ei = np.asarray(edge_index)
    r = ei[0].astype(np.int64)
    c = ei[1].astype(np.int64)
    ok = (r >= 0) & (r < N) & (c >= 0) & (c < N)
    r, c = r[ok], c[ok]
    key = np.unique(r * N + c)
    r, c = key // N, key % N

    core = r // R
    half = (r % R) // HALF
    moff = (r % R) % HALF
    t = c // P
    p = c % P
    col = half * NT + t
    cell = (core * P + p) * (2 * NT) + col

    order = np.argsort(cell, kind="stable")
    cell_s = cell[order]
    moff_s = moff[order]
    idx = np.arange(len(cell_s))
    first = np.ones(len(cell_s), dtype=bool)
    if len(cell_s) > 1:
        first[1:] = cell_s[1:] != cell_s[:-1]
    start = np.where(first, idx, 0)
    np.maximum.accumulate(start, out=start)
    slot = idx - start

    kmax = int(slot.max()) + 1 if len(slot) else 1
    kpad = max(2, (kmax + 1) // 2 * 2)

    si_all = np.full((NCORES, P, 2 * NT, kpad), -1, dtype=np.int16)
    core_s = cell_s // (P * 2 * NT)
    p_s = (cell_s // (2 * NT)) % P
    col_s = cell_s % (2 * NT)
    si_all[core_s, p_s, col_s, slot] = moff_s.astype(np.int16)
    return si_all, kpad


def make_in_maps(x, edge_index):
    x = np.asarray(x, dtype=np.float32)
    si_all, kpad = _prep_mask(edge_index)
    xbf = x.astype(ml_dtypes.bfloat16)
    in_maps = []
    for k in range(NCORES):
        in_maps.append({
            "xk": np.ascontiguousarray(xbf[k * R:(k + 1) * R]),
            "si": np.ascontiguousarray(si_all[k]),
        })
    return in_maps, kpad


def run(x, edge_index, trace=False):
    from concourse.bass_utils import run_bass_kernel_spmd

    in_maps, kpad = make_in_maps(x, edge_index)
    nc = get_program(kpad)
    br = run_bass_kernel_spmd(nc, in_maps, list(range(NCORES)), trace=trace)
    out = np.concatenate(
        [br.results[k]["out"] for k in range(NCORES)], axis=0
    ).astype(np.float32)
    return out, br


def kernel(x, edge_index):
    out, _ = run(x, edge_index, trace=False)
    return out
